# revision 3
# baseline (speedup 1.0000x reference)
"""Multi-head attention (B=2, S=2048, H=1024, 16 heads) on 8 TRN2 NeuronCores.

Sharding: tensor-parallel heads x data-parallel batch. Core c -> batch c//4,
head group c%4 (4 heads each). Megatron-style partial out-projections summed
on the host.

V2 design (vs the phase-serial baseline):
- Single stacked head-pair layout for Q^T/K^T ([128, S] per ci: partitions
  0:64 = even head, 64:128 = odd head). Scores matmuls read the matching
  64-partition band directly (base_partition 0 or 64), so no duplication
  DMAs and only one PSUM->SBUF copy per projection half.
- V^T -> natural V via DMA xbar transposes (sync/scalar queues) instead of
  PE transpose + DVE strided copies.
- j-outer attention pipeline: for each 512-query block, all 4 heads run
  scores -> exp -> ctx -> division, then the out-projection for that block
  streams immediately; output DMA overlaps the next block.
- Softmax exp split between the ACT engine (real exp) and the DVE
  (Schraudolph fast-exp: int16 bits = s*a + b, bitcast as bf16), keeping
  both engines below the PE's critical path.
- bk dropped: adding bk shifts every score of a query equally, which softmax
  cancels (bv/bo are folded into a host-side additive constant; bq is
  applied on-device).
"""

import ml_dtypes
import numpy as np

import concourse.bacc as bacc
import concourse.mybir as mybir
import concourse.tile as tile
from concourse.bass_utils import run_bass_kernel_spmd

NCORES = 8
B, S, HID = 2, 2048, 1024
NH, HD = 16, 64
HPC = 4            # heads per core
QC = HPC * HD      # 256 local projection cols per core
HC = HID // 128    # 8 hidden chunks
TC = S // 128      # 16 token chunks
TB = S // 512      # 4 query blocks
NCP = TC // 2      # 8 chunk-pairs per head

F32 = mybir.dt.float32
BF16 = mybir.dt.bfloat16
FP16 = mybir.dt.float16
I16 = mybir.dt.int16
EXP = mybir.ActivationFunctionType.Exp
MULT = mybir.AluOpType.mult
ADD = mybir.AluOpType.add

# Schraudolph fast-exp (bf16): bits = round(s * A + B), bitcast bf16 ~ exp(s).
# B includes a -7.0 correction that centers the mean multiplicative error at
# ~1.002 so ACT-exp and DVE-fast-exp tiles mix consistently in the softmax.
SCH_A = 128.0 / float(np.log(2.0))
SCH_B = 16256.0 - 7.0
# chunk-pairs handled by the DVE fast-exp (rest on ACT): 3/8 of the tiles
SCH_CPS = (2, 5, 7)


def build_nc():
    nc = bacc.Bacc("TRN2", target_bir_lowering=False, debug=False,
                   num_devices=NCORES)
    xT = nc.declare_dram_parameter("xT", [HID, S], FP16, isOutput=False)
    wq = nc.declare_dram_parameter("wq", [HID, QC], FP16, isOutput=False)
    wk = nc.declare_dram_parameter("wk", [HID, QC], FP16, isOutput=False)
    wv = nc.declare_dram_parameter("wv", [HID, QC], FP16, isOutput=False)
    wo = nc.declare_dram_parameter("wo", [QC, HID], BF16, isOutput=False)
    bq = nc.declare_dram_parameter("bq", [QC], F32, isOutput=False)
    out = nc.declare_dram_parameter("out", [S, HID], BF16, isOutput=True)

    with tile.TileContext(nc) as tc:
        with (
            tc.tile_pool(name="const", bufs=1) as constp,
            tc.tile_pool(name="qkv", bufs=1) as qkvp,
        ):
            wo_sb = constp.tile([128, 2 * HID], BF16)
            bq_sb = constp.tile([128, 2], F32)
            # Q^T/K^T per ci: [128, S] with even head on partitions 0:64,
            # odd head on 64:128.
            qt = qkvp.tile([128, 2 * S], FP16)
            kt = qkvp.tile([128, 2 * S], FP16)
            vt_sb = qkvp.tile([128, 2 * S], BF16)
            # Natural V strips: strip (t*HPC + h) at col offset *128,
            # V in cols 0:64, ones in col 64 (softmax denominator trick).
            v_sb = qkvp.tile([128, TC * HPC * 128], BF16)
            ctxf_sb = qkvp.tile([128, 2 * S], BF16)

            # ones column of every strip in one strided memset
            ones_ap = v_sb[:, :].rearrange("p (s e) -> p s e", e=128)[:, :, HD:HD + 1]
            nc.vector.memset(ones_ap, 1.0)

            # ---- phase A: projections ------------------------------------
            with (
                tc.tile_pool(name="xw", bufs=1) as xwp,
                tc.tile_pool(name="ps1", bufs=2, space="PSUM") as ps1,
            ):
                xT_sb = xwp.tile([128, HC * S], FP16)
                wq_sb = xwp.tile([128, HC * QC], FP16)
                wk_sb = xwp.tile([128, HC * QC], FP16)
                wv_sb = xwp.tile([128, HC * QC], FP16)

                xt_dmas = {}
                for hc in range(HC):
                    r = slice(hc * 128, (hc + 1) * 128)
                    eng = nc.sync if hc % 2 == 0 else nc.scalar
                    if hc == 0:
                        for j in range(TB):
                            xt_dmas[hc] = eng.dma_start(
                                xT_sb[:, j * 512:(j + 1) * 512],
                                xT[r, j * 512:(j + 1) * 512])
                    else:
                        xt_dmas[hc] = eng.dma_start(
                            xT_sb[:, hc * S:(hc + 1) * S], xT[r, :])
                    nc.scalar.dma_start(wq_sb[:, hc * QC:(hc + 1) * QC],
                                        wq[r, :])
                    nc.sync.dma_start(wk_sb[:, hc * QC:(hc + 1) * QC],
                                      wk[r, :])
                for ci in range(2):
                    nc.sync.dma_start(bq_sb[:, ci:ci + 1],
                                      bq[ci * 128:(ci + 1) * 128])

                # Q then K per ci so scores for heads 0/1 unblock first.
                qk_mms = {}
                for name, w_sb, dst in (("q", wq_sb, qt), ("k", wk_sb, kt)):
                    for ci in range(2):
                        ps = ps1.tile([128, S], F32, tag="ps1")
                        for hc in range(HC):
                            for j in range(TB):
                                mm = nc.tensor.matmul(
                                    ps[:, j * 512:(j + 1) * 512],
                                    w_sb[:, hc * QC + ci * 128:
                                         hc * QC + ci * 128 + 128],
                                    xT_sb[:, hc * S + j * 512:
                                          hc * S + j * 512 + 512],
                                    start=(hc == 0), stop=(hc == HC - 1))
                                qk_mms[(name, ci, hc, j)] = mm
                        if name == "q":
                            nc.vector.tensor_scalar_add(
                                dst[:, ci * S:(ci + 1) * S], ps[:, :],
                                bq_sb[:, ci:ci + 1])
                        else:
                            nc.vector.tensor_copy(
                                dst[:, ci * S:(ci + 1) * S], ps[:, :])

                # pace later input loads behind early Q matmuls
                wv_dmas = {}
                for hc in range(HC):
                    r = slice(hc * 128, (hc + 1) * 128)
                    d = nc.scalar.dma_start(
                        wv_sb[:, hc * QC:(hc + 1) * QC], wv[r, :])
                    wv_dmas[hc] = d
                    tile.add_dep_helper(d.ins, qk_mms[("q", 0, hc, 1)].ins,
                                        reason="pace wv load")
                for hc in range(2, HC):
                    tile.add_dep_helper(xt_dmas[hc].ins,
                                        qk_mms[("q", 0, hc - 2, 3)].ins,
                                        reason="pace xT load")
                for ci in range(2):
                    d = nc.sync.dma_start(
                        wo_sb[:, ci * HID:(ci + 1) * HID],
                        wo[ci * 128:(ci + 1) * 128, :])
                    tile.add_dep_helper(d.ins, qk_mms[("k", ci, 5, 1)].ins,
                                        reason="pace wo load")

                vt_copies = {}
                for ci in range(2):
                    ps = ps1.tile([128, S], F32, tag="ps1")
                    for hc in range(HC):
                        for j in range(TB):
                            nc.tensor.matmul(
                                ps[:, j * 512:(j + 1) * 512],
                                wv_sb[:, hc * QC + ci * 128:
                                      hc * QC + ci * 128 + 128],
                                xT_sb[:, hc * S + j * 512:
                                      hc * S + j * 512 + 512],
                                start=(hc == 0), stop=(hc == HC - 1))
                    vt_copies[ci] = nc.vector.tensor_copy(
                        vt_sb[:, ci * S:(ci + 1) * S], ps[:, :])

            # V^T -> natural V via DMA xbar transposes (no PE/DVE cost).
            for ci in range(2):
                for half in range(2):
                    h = 2 * ci + half
                    eng = nc.sync if h % 2 == 0 else nc.scalar
                    for t in range(TC):
                        eng.dma_start_transpose(
                            v_sb[:, (t * HPC + h) * 128:
                                 (t * HPC + h) * 128 + HD],
                            vt_sb[half * 64:half * 64 + 64,
                                  ci * S + t * 128:ci * S + (t + 1) * 128])

            # ---- phase B: attention, j-outer pipeline --------------------
            with (
                tc.tile_pool(name="bigps", bufs=3, space="PSUM") as bigps,
                tc.tile_pool(name="ctps", bufs=2, space="PSUM") as ctps,
                tc.tile_pool(name="probs", bufs=5) as probsp,
                tc.tile_pool(name="div", bufs=2) as divp,
                tc.tile_pool(name="ostg", bufs=3) as ostg,
            ):
                stages = [(j, h, cp) for j in range(TB) for h in range(HPC)
                          for cp in range(NCP)]
                probs_tiles = {}
                ctx_tiles = {}

                def emit_scores(j, h, cp):
                    ci, half = h // 2, h % 2
                    pb = slice(half * 64, half * 64 + 64)
                    sp = bigps.tile([128, 1024], F32, tag="big")
                    probs_c = probsp.tile([128, 1024], BF16, tag="probs",
                                          name=f"probs_j{j}h{h}cp{cp}")
                    probs_tiles[(j, h, cp)] = probs_c
                    for ck in range(2):
                        c = 2 * cp + ck
                        nc.tensor.matmul(
                            sp[:, ck * 512:(ck + 1) * 512],
                            kt[pb, ci * S + c * 128:ci * S + (c + 1) * 128],
                            qt[pb, ci * S + j * 512:ci * S + (j + 1) * 512],
                            start=True, stop=True)
                    if cp in SCH_CPS:
                        nc.vector.tensor_scalar(
                            probs_c[:, :].bitcast(I16), sp[:, :],
                            SCH_A, SCH_B, op0=MULT, op1=ADD)
                    else:
                        nc.scalar.activation(probs_c[:, :], sp[:, :], EXP)

                def emit_ctx(j, h, cp):
                    if cp == 0:
                        ctx_tiles[(j, h)] = ctps.tile(
                            [65, 512], F32, tag="ctx", name=f"ctx_j{j}h{h}")
                    ctx_ps = ctx_tiles[(j, h)]
                    probs_c = probs_tiles.pop((j, h, cp))
                    for ck in range(2):
                        c = 2 * cp + ck
                        vbase = (c * HPC + h) * 128
                        nc.tensor.matmul(
                            ctx_ps[0:65, :],
                            v_sb[:, vbase:vbase + 65],
                            probs_c[:, ck * 512:(ck + 1) * 512],
                            start=(cp == 0 and ck == 0),
                            stop=(cp == NCP - 1 and ck == 1))

                def emit_division(j, h):
                    ci, half = h // 2, h % 2
                    ctx_ps = ctx_tiles.pop((j, h))
                    drow = divp.tile([1, 512], F32, tag="drow")
                    nc.scalar.copy(drow[0:1, :], ctx_ps[64:65, :])
                    denr = divp.tile([128, 4], F32, tag="denr")
                    nc.sync.dma_start(denr[:, :], drow[0:1, :])
                    recr = divp.tile([128, 4], F32, tag="recr")
                    nc.vector.reciprocal(recr[:], denr[:])
                    rrow = divp.tile([1, 512], F32, tag="rrow")
                    nc.sync.dma_start(rrow[:, :], recr[:, :])
                    Dt = divp.tile([64, 512], F32, tag="Dt")
                    nc.gpsimd.partition_broadcast(Dt[:, :], rrow[0:1, :])
                    dst_cols = slice(ci * S + j * 512, ci * S + (j + 1) * 512)
                    if half == 0:
                        nc.vector.tensor_tensor(
                            out=ctxf_sb[0:64, dst_cols],
                            in0=ctx_ps[0:64, :], in1=Dt[0:64, :], op=MULT)
                    else:
                        ctxd = divp.tile([64, 512], BF16, tag="ctxd")
                        nc.vector.tensor_tensor(
                            out=ctxd[0:64, :],
                            in0=ctx_ps[0:64, :], in1=Dt[0:64, :], op=MULT)
                        nc.scalar.dma_start(ctxf_sb[64:128, dst_cols],
                                            ctxd[0:64, :])

                def emit_outproj(j):
                    for t in range(4 * j, 4 * j + 4):
                        op = bigps.tile([128, 1024], F32, tag="big")
                        for ci in range(2):
                            for oc in range(2):
                                nc.tensor.matmul(
                                    op[:, oc * 512:(oc + 1) * 512],
                                    ctxf_sb[:, ci * S + t * 128:
                                            ci * S + t * 128 + 128],
                                    wo_sb[:, ci * HID + oc * 512:
                                          ci * HID + oc * 512 + 512],
                                    start=(ci == 0), stop=(ci == 1))
                        ot = ostg.tile([128, 1024], BF16, tag="ot")
                        nc.scalar.copy(ot[:, 0:512], op[:, 0:512])
                        nc.vector.tensor_copy(ot[:, 512:1024], op[:, 512:1024])
                        nc.sync.dma_start(out[t * 128:(t + 1) * 128, :],
                                          ot[:, :])

                LEAD = 2
                for i in range(len(stages) + LEAD):
                    if i < len(stages):
                        emit_scores(*stages[i])
                    if i >= LEAD:
                        j, h, cp = stages[i - LEAD]
                        emit_ctx(j, h, cp)
                        if cp == NCP - 1:
                            emit_division(j, h)
                            if h == HPC - 1:
                                emit_outproj(j)

    nc.compile()
    return nc


_NC = None


def _get_nc():
    global _NC
    if _NC is None:
        _NC = build_nc()
    return _NC


def make_in_maps(x, Wq, bq, Wk, bk, Wv, bv, Wo, bo):
    in_maps = []
    for core in range(NCORES):
        b, g = core // 4, core % 4
        sl = slice(g * QC, (g + 1) * QC)
        in_maps.append({
            "xT": np.ascontiguousarray(x[b].T).astype(np.float16),
            "wq": (np.ascontiguousarray(Wq[:, sl]) * 0.125).astype(np.float16),
            "wk": np.ascontiguousarray(Wk[:, sl]).astype(np.float16),
            "wv": np.ascontiguousarray(Wv[:, sl]).astype(np.float16),
            "wo": np.ascontiguousarray(Wo[sl, :]).astype(ml_dtypes.bfloat16),
            "bq": (np.asarray(bq[sl]) * 0.125).astype(np.float32),
        })
    return in_maps


def combine_outputs(core_outs, Wv_bias_term):
    full = np.empty((B, S, HID), np.float32)
    for b in range(B):
        acc = core_outs[4 * b].astype(np.float32).copy()
        for g in range(1, 4):
            acc += core_outs[4 * b + g]
        full[b] = acc + Wv_bias_term
    return full


def kernel(**inputs):
    x = np.asarray(inputs["x"], np.float32)
    Wq = np.asarray(inputs["Wq"], np.float32)
    bq = np.asarray(inputs["bq"], np.float32)
    Wk = np.asarray(inputs["Wk"], np.float32)
    bk = np.asarray(inputs["bk"], np.float32)
    Wv = np.asarray(inputs["Wv"], np.float32)
    bv = np.asarray(inputs["bv"], np.float32)
    Wo = np.asarray(inputs["Wo"], np.float32)
    bo = np.asarray(inputs["bo"], np.float32)

    nc = _get_nc()
    in_maps = make_in_maps(x, Wq, bq, Wk, bk, Wv, bv, Wo, bo)
    res = run_bass_kernel_spmd(nc, in_maps, core_ids=list(range(NCORES)))
    core_outs = [res.results[c]["out"] for c in range(NCORES)]
    bias_term = (bv @ Wo + bo).astype(np.float32)
    return combine_outputs(core_outs, bias_term)


# revision 4
# speedup vs baseline: 1.1624x; 1.1624x over previous
"""Multi-head attention (B=2, S=2048, H=1024, 16 heads) on 8 TRN2 NeuronCores.

Sharding: tensor-parallel heads x data-parallel batch. Core c -> batch c//4,
head group c%4 (4 heads each). Megatron-style partial out-projections summed
on the host.

V2 design (vs the phase-serial baseline):
- Single stacked head-pair layout for Q^T/K^T ([128, S] per ci: partitions
  0:64 = even head, 64:128 = odd head). Scores matmuls read the matching
  64-partition band directly (base_partition 0 or 64), so no duplication
  DMAs and only one PSUM->SBUF copy per projection half.
- V projected FIRST, then V^T -> natural V via DMA xbar transposes
  (sync/scalar queues, h-major order) overlapping the Q/K projections, so
  V strips are ready before the first ctx matmul needs them.
- j-outer attention pipeline: for each 512-query block, all 4 heads run
  scores -> exp -> ctx -> division; the block's out-projection is emitted a
  few pipeline steps later (hiding division latency) and streams its DMA.
- Softmax exp split between the ACT engine (real exp) and the DVE
  (Schraudolph fast-exp: int16 bits = s*a + b, bitcast as bf16), keeping
  both engines below the PE's critical path.
- bk dropped: adding bk shifts every score of a query equally, which softmax
  cancels (bv/bo are folded into a host-side additive constant; bq is
  applied on-device).
"""

import ml_dtypes
import numpy as np

import concourse.bacc as bacc
import concourse.mybir as mybir
import concourse.tile as tile
from concourse.bass_utils import run_bass_kernel_spmd

NCORES = 8
B, S, HID = 2, 2048, 1024
NH, HD = 16, 64
HPC = 4            # heads per core
QC = HPC * HD      # 256 local projection cols per core
HC = HID // 128    # 8 hidden chunks
TC = S // 128      # 16 token chunks
TB = S // 512      # 4 query blocks
NCP = TC // 2      # 8 chunk-pairs per head

F32 = mybir.dt.float32
BF16 = mybir.dt.bfloat16
FP16 = mybir.dt.float16
I16 = mybir.dt.int16
EXP = mybir.ActivationFunctionType.Exp
MULT = mybir.AluOpType.mult
ADD = mybir.AluOpType.add

# Schraudolph fast-exp (bf16): bits = round(s * A + B), bitcast bf16 ~ exp(s).
# B includes a -7.0 correction that centers the mean multiplicative error at
# ~1.002 so ACT-exp and DVE-fast-exp tiles mix consistently in the softmax.
SCH_A = 128.0 / float(np.log(2.0))
SCH_B = 16256.0 - 7.0
# chunk-pairs handled by the DVE fast-exp (rest on ACT): 3/8 of the tiles
SCH_CPS = (2, 5, 7)
# out-projection trails the last division by this many pipeline steps
OP_DELAY = 6


def build_nc():
    nc = bacc.Bacc("TRN2", target_bir_lowering=False, debug=False,
                   num_devices=NCORES)
    xT = nc.declare_dram_parameter("xT", [HID, S], FP16, isOutput=False)
    wq = nc.declare_dram_parameter("wq", [HID, QC], FP16, isOutput=False)
    wk = nc.declare_dram_parameter("wk", [HID, QC], FP16, isOutput=False)
    wv = nc.declare_dram_parameter("wv", [HID, QC], FP16, isOutput=False)
    wo = nc.declare_dram_parameter("wo", [QC, HID], BF16, isOutput=False)
    bq = nc.declare_dram_parameter("bq", [QC], F32, isOutput=False)
    out = nc.declare_dram_parameter("out", [S, HID], BF16, isOutput=True)

    with tile.TileContext(nc) as tc:
        with (
            tc.tile_pool(name="const", bufs=1) as constp,
            tc.tile_pool(name="qkv", bufs=1) as qkvp,
        ):
            wo_sb = constp.tile([128, 2 * HID], BF16)
            bq_sb = constp.tile([128, 2], F32)
            # Q^T/K^T per ci: [128, S] with even head on partitions 0:64,
            # odd head on 64:128.
            qt = qkvp.tile([128, 2 * S], FP16)
            kt = qkvp.tile([128, 2 * S], FP16)
            vt_sb = qkvp.tile([128, 2 * S], BF16)
            # Natural V strips: strip (t*HPC + h) at col offset *128,
            # V in cols 0:64, ones in col 64 (softmax denominator trick).
            v_sb = qkvp.tile([128, TC * HPC * 128], BF16)
            ctxf_sb = qkvp.tile([128, 2 * S], BF16)

            # ones column of every strip in one strided memset
            ones_ap = v_sb[:, :].rearrange("p (s e) -> p s e", e=128)[:, :, HD:HD + 1]
            nc.vector.memset(ones_ap, 1.0)

            # ---- phase A: projections (V first) --------------------------
            with (
                tc.tile_pool(name="xw", bufs=1) as xwp,
                tc.tile_pool(name="ps1", bufs=2, space="PSUM") as ps1,
            ):
                xT_sb = xwp.tile([128, HC * S], FP16)
                wq_sb = xwp.tile([128, HC * QC], FP16)
                wk_sb = xwp.tile([128, HC * QC], FP16)
                wv_sb = xwp.tile([128, HC * QC], FP16)

                xt_dmas = {}
                for hc in range(HC):
                    r = slice(hc * 128, (hc + 1) * 128)
                    eng = nc.sync if hc % 2 == 0 else nc.scalar
                    if hc == 0:
                        for j in range(TB):
                            xt_dmas[hc] = eng.dma_start(
                                xT_sb[:, j * 512:(j + 1) * 512],
                                xT[r, j * 512:(j + 1) * 512])
                    else:
                        xt_dmas[hc] = eng.dma_start(
                            xT_sb[:, hc * S:(hc + 1) * S], xT[r, :])
                    nc.scalar.dma_start(wv_sb[:, hc * QC:(hc + 1) * QC],
                                        wv[r, :])
                for ci in range(2):
                    nc.sync.dma_start(bq_sb[:, ci:ci + 1],
                                      bq[ci * 128:(ci + 1) * 128])

                # V projection first; its PSUM->SBUF copies unblock the DMA
                # xbar transposes which then overlap the Q/K projections.
                v_mms = {}
                for ci in range(2):
                    ps = ps1.tile([128, S], F32, tag="ps1")
                    for hc in range(HC):
                        for j in range(TB):
                            v_mms[(ci, hc, j)] = nc.tensor.matmul(
                                ps[:, j * 512:(j + 1) * 512],
                                wv_sb[:, hc * QC + ci * 128:
                                      hc * QC + ci * 128 + 128],
                                xT_sb[:, hc * S + j * 512:
                                      hc * S + j * 512 + 512],
                                start=(hc == 0), stop=(hc == HC - 1))
                    nc.vector.tensor_copy(
                        vt_sb[:, ci * S:(ci + 1) * S], ps[:, :])

                # pace the Q/K weight loads behind early V matmuls
                for hc in range(HC):
                    r = slice(hc * 128, (hc + 1) * 128)
                    d = nc.scalar.dma_start(
                        wq_sb[:, hc * QC:(hc + 1) * QC], wq[r, :])
                    tile.add_dep_helper(d.ins, v_mms[(0, hc, 1)].ins,
                                        reason="pace wq load")
                    d = nc.sync.dma_start(
                        wk_sb[:, hc * QC:(hc + 1) * QC], wk[r, :])
                    tile.add_dep_helper(d.ins, v_mms[(0, hc, 2)].ins,
                                        reason="pace wk load")
                for hc in range(2, HC):
                    tile.add_dep_helper(xt_dmas[hc].ins,
                                        v_mms[(0, hc - 2, 3)].ins,
                                        reason="pace xT load")

                # V^T -> natural V, h-major so head 0's strips land first.
                # Emitted inside phase A right after the vt copies so the
                # DMA queues start them as early as possible.
                for ci in range(2):
                    for half in range(2):
                        h = 2 * ci + half
                        eng = nc.sync if half == 0 else nc.scalar
                        for t in range(TC):
                            eng.dma_start_transpose(
                                v_sb[:, (t * HPC + h) * 128:
                                     (t * HPC + h) * 128 + HD],
                                vt_sb[half * 64:half * 64 + 64,
                                      ci * S + t * 128:ci * S + (t + 1) * 128])

                # Q/K projections: ci0 pair first so heads 0/1 unblock.
                qk_mms = {}
                for ci in range(2):
                    for name, w_sb, dst in (("q", wq_sb, qt), ("k", wk_sb, kt)):
                        ps = ps1.tile([128, S], F32, tag="ps1")
                        for hc in range(HC):
                            for j in range(TB):
                                qk_mms[(name, ci, hc, j)] = nc.tensor.matmul(
                                    ps[:, j * 512:(j + 1) * 512],
                                    w_sb[:, hc * QC + ci * 128:
                                         hc * QC + ci * 128 + 128],
                                    xT_sb[:, hc * S + j * 512:
                                          hc * S + j * 512 + 512],
                                    start=(hc == 0), stop=(hc == HC - 1))
                        if name == "q":
                            nc.vector.tensor_scalar_add(
                                dst[:, ci * S:(ci + 1) * S], ps[:, :],
                                bq_sb[:, ci:ci + 1])
                        else:
                            nc.vector.tensor_copy(
                                dst[:, ci * S:(ci + 1) * S], ps[:, :])

                for ci in range(2):
                    d = nc.sync.dma_start(
                        wo_sb[:, ci * HID:(ci + 1) * HID],
                        wo[ci * 128:(ci + 1) * 128, :])
                    tile.add_dep_helper(d.ins, qk_mms[("q", ci, 5, 1)].ins,
                                        reason="pace wo load")

            # ---- phase B: attention, j-outer pipeline --------------------
            with (
                tc.tile_pool(name="bigps", bufs=3, space="PSUM") as bigps,
                tc.tile_pool(name="ctps", bufs=2, space="PSUM") as ctps,
                tc.tile_pool(name="probs", bufs=5) as probsp,
                tc.tile_pool(name="div", bufs=2) as divp,
                tc.tile_pool(name="ostg", bufs=3) as ostg,
            ):
                stages = [(j, h, cp) for j in range(TB) for h in range(HPC)
                          for cp in range(NCP)]
                probs_tiles = {}
                ctx_tiles = {}

                def emit_scores(j, h, cp):
                    ci, half = h // 2, h % 2
                    pb = slice(half * 64, half * 64 + 64)
                    sp = bigps.tile([128, 1024], F32, tag="big")
                    probs_c = probsp.tile([128, 1024], BF16, tag="probs",
                                          name=f"probs_j{j}h{h}cp{cp}")
                    probs_tiles[(j, h, cp)] = probs_c
                    for ck in range(2):
                        c = 2 * cp + ck
                        nc.tensor.matmul(
                            sp[:, ck * 512:(ck + 1) * 512],
                            kt[pb, ci * S + c * 128:ci * S + (c + 1) * 128],
                            qt[pb, ci * S + j * 512:ci * S + (j + 1) * 512],
                            start=True, stop=True)
                    if cp in SCH_CPS:
                        nc.vector.tensor_scalar(
                            probs_c[:, :].bitcast(I16), sp[:, :],
                            SCH_A, SCH_B, op0=MULT, op1=ADD)
                    else:
                        nc.scalar.activation(probs_c[:, :], sp[:, :], EXP)

                def emit_ctx(j, h, cp):
                    if cp == 0:
                        ctx_tiles[(j, h)] = ctps.tile(
                            [65, 512], F32, tag="ctx", name=f"ctx_j{j}h{h}")
                    ctx_ps = ctx_tiles[(j, h)]
                    probs_c = probs_tiles.pop((j, h, cp))
                    for ck in range(2):
                        c = 2 * cp + ck
                        vbase = (c * HPC + h) * 128
                        nc.tensor.matmul(
                            ctx_ps[0:65, :],
                            v_sb[:, vbase:vbase + 65],
                            probs_c[:, ck * 512:(ck + 1) * 512],
                            start=(cp == 0 and ck == 0),
                            stop=(cp == NCP - 1 and ck == 1))

                def emit_division(j, h):
                    ci, half = h // 2, h % 2
                    ctx_ps = ctx_tiles.pop((j, h))
                    drow = divp.tile([1, 512], F32, tag="drow")
                    nc.scalar.copy(drow[0:1, :], ctx_ps[64:65, :])
                    rrow = divp.tile([1, 512], F32, tag="rrow")
                    nc.vector.reciprocal(rrow[0:1, :], drow[0:1, :])
                    Dt = divp.tile([64, 512], F32, tag="Dt")
                    nc.gpsimd.partition_broadcast(Dt[:, :], rrow[0:1, :])
                    dst_cols = slice(ci * S + j * 512, ci * S + (j + 1) * 512)
                    if half == 0:
                        nc.vector.tensor_tensor(
                            out=ctxf_sb[0:64, dst_cols],
                            in0=ctx_ps[0:64, :], in1=Dt[0:64, :], op=MULT)
                    else:
                        ctxd = divp.tile([64, 512], BF16, tag="ctxd")
                        nc.vector.tensor_tensor(
                            out=ctxd[0:64, :],
                            in0=ctx_ps[0:64, :], in1=Dt[0:64, :], op=MULT)
                        nc.scalar.dma_start(ctxf_sb[64:128, dst_cols],
                                            ctxd[0:64, :])

                def emit_outproj_t(t):
                    j = t // 4
                    op = bigps.tile([128, 1024], F32, tag="big")
                    for ci in range(2):
                        for oc in range(2):
                            nc.tensor.matmul(
                                op[:, oc * 512:(oc + 1) * 512],
                                ctxf_sb[:, ci * S + t * 128:
                                        ci * S + t * 128 + 128],
                                wo_sb[:, ci * HID + oc * 512:
                                      ci * HID + oc * 512 + 512],
                                start=(ci == 0), stop=(ci == 1))
                    ot = ostg.tile([128, 1024], BF16, tag="ot")
                    nc.scalar.copy(ot[:, 0:512], op[:, 0:512])
                    nc.vector.tensor_copy(ot[:, 512:1024], op[:, 512:1024])
                    nc.sync.dma_start(out[t * 128:(t + 1) * 128, :], ot[:, :])

                LEAD = 2
                op_due = {}      # step index -> list of t tiles to emit
                nsteps = len(stages) + LEAD + OP_DELAY + 4
                for i in range(nsteps):
                    if i < len(stages):
                        emit_scores(*stages[i])
                    if LEAD <= i < len(stages) + LEAD:
                        j, h, cp = stages[i - LEAD]
                        emit_ctx(j, h, cp)
                        if cp == NCP - 1:
                            emit_division(j, h)
                            if h == HPC - 1:
                                for tt in range(4):
                                    op_due.setdefault(
                                        i + OP_DELAY + tt, []).append(4 * j + tt)
                    for t in op_due.pop(i, ()):
                        emit_outproj_t(t)

    nc.compile()
    return nc


_NC = None


def _get_nc():
    global _NC
    if _NC is None:
        _NC = build_nc()
    return _NC


def make_in_maps(x, Wq, bq, Wk, bk, Wv, bv, Wo, bo):
    in_maps = []
    for core in range(NCORES):
        b, g = core // 4, core % 4
        sl = slice(g * QC, (g + 1) * QC)
        in_maps.append({
            "xT": np.ascontiguousarray(x[b].T).astype(np.float16),
            "wq": (np.ascontiguousarray(Wq[:, sl]) * 0.125).astype(np.float16),
            "wk": np.ascontiguousarray(Wk[:, sl]).astype(np.float16),
            "wv": np.ascontiguousarray(Wv[:, sl]).astype(np.float16),
            "wo": np.ascontiguousarray(Wo[sl, :]).astype(ml_dtypes.bfloat16),
            "bq": (np.asarray(bq[sl]) * 0.125).astype(np.float32),
        })
    return in_maps


def combine_outputs(core_outs, Wv_bias_term):
    full = np.empty((B, S, HID), np.float32)
    for b in range(B):
        acc = core_outs[4 * b].astype(np.float32).copy()
        for g in range(1, 4):
            acc += core_outs[4 * b + g]
        full[b] = acc + Wv_bias_term
    return full


def kernel(**inputs):
    x = np.asarray(inputs["x"], np.float32)
    Wq = np.asarray(inputs["Wq"], np.float32)
    bq = np.asarray(inputs["bq"], np.float32)
    Wk = np.asarray(inputs["Wk"], np.float32)
    bk = np.asarray(inputs["bk"], np.float32)
    Wv = np.asarray(inputs["Wv"], np.float32)
    bv = np.asarray(inputs["bv"], np.float32)
    Wo = np.asarray(inputs["Wo"], np.float32)
    bo = np.asarray(inputs["bo"], np.float32)

    nc = _get_nc()
    in_maps = make_in_maps(x, Wq, bq, Wk, bk, Wv, bv, Wo, bo)
    res = run_bass_kernel_spmd(nc, in_maps, core_ids=list(range(NCORES)))
    core_outs = [res.results[c]["out"] for c in range(NCORES)]
    bias_term = (bv @ Wo + bo).astype(np.float32)
    return combine_outputs(core_outs, bias_term)


# revision 7
# speedup vs baseline: 1.5123x; 1.3011x over previous
"""Multi-head attention (B=2, S=2048, H=1024, 16 heads) on 8 TRN2 NeuronCores.

Sharding: tensor-parallel heads x data-parallel batch. Core c -> batch c//4,
head group c%4 (4 heads each). Megatron-style partial out-projections summed
on the host.

V3 design:
- Projections emit V(ci)->Q(ci)->K(ci) so the V^T -> natural-V DMA xbar
  transposes (h-major, both queues) overlap the Q/K projections and the
  first ctx matmuls never stall on V strips.
- Q^T/K^T stored stacked per ci ([128, S]: even head on partitions 0:64,
  odd on 64:128) with band-swapped duplicates (qtX/ktX via SBUF DMA):
  each scores chunk-pair runs as two concurrent row-tiled matmuls on
  opposite partition bands (the PE overlaps them, 2x throughput).
- j-outer attention pipeline per 512-query block; ctx accumulates
  alternately into two PSUM banks (A: even chunks, B: odd chunks) to dodge
  same-bank accumulation turnaround; division starts with craw = A + B,
  which also frees the PSUM banks quickly.
- Softmax denominator via the ones-column row-64 trick; reciprocal on a
  [128, 4] DMA-gathered layout (a [1, 512] single-partition reciprocal is
  ~6x slower than the whole gather chain).
- exp split: ACT does 5/8 of tiles (real exp), DVE does 3/8 via Schraudolph
  fast-exp (int16 bits = s*a + b bitcast as bf16, bias tuned so the mean
  multiplicative error matches exp for consistent mixing).
- bk dropped: it shifts every score of a query equally and softmax cancels
  it. bv/bo are folded into a host-side additive constant; bq applied
  on-device.
"""

import ml_dtypes
import numpy as np

import concourse.bacc as bacc
import concourse.mybir as mybir
import concourse.tile as tile
from concourse.bass_utils import run_bass_kernel_spmd

NCORES = 8
B, S, HID = 2, 2048, 1024
NH, HD = 16, 64
HPC = 4            # heads per core
QC = HPC * HD      # 256 local projection cols per core
HC = HID // 128    # 8 hidden chunks
TC = S // 128      # 16 token chunks
TB = S // 512      # 4 query blocks
NCP = TC // 2      # 8 chunk-pairs per head

F32 = mybir.dt.float32
BF16 = mybir.dt.bfloat16
FP16 = mybir.dt.float16
I16 = mybir.dt.int16
EXP = mybir.ActivationFunctionType.Exp
MULT = mybir.AluOpType.mult
ADD = mybir.AluOpType.add
TADD = mybir.AluOpType.add

SCH_A = 128.0 / float(np.log(2.0))
SCH_B = 16256.0 - 7.0
SCH_CPS = (2, 5, 7)
OP_DELAY = 6


def build_nc():
    nc = bacc.Bacc("TRN2", target_bir_lowering=False, debug=False,
                   num_devices=NCORES)
    xT = nc.declare_dram_parameter("xT", [HID, S], FP16, isOutput=False)
    wq = nc.declare_dram_parameter("wq", [HID, QC], FP16, isOutput=False)
    wk = nc.declare_dram_parameter("wk", [HID, QC], FP16, isOutput=False)
    wv = nc.declare_dram_parameter("wv", [HID, QC], FP16, isOutput=False)
    wo = nc.declare_dram_parameter("wo", [QC, HID], BF16, isOutput=False)
    bq = nc.declare_dram_parameter("bq", [QC], F32, isOutput=False)
    out = nc.declare_dram_parameter("out", [S, HID], BF16, isOutput=True)

    with tile.TileContext(nc) as tc:
        with (
            tc.tile_pool(name="const", bufs=1) as constp,
            tc.tile_pool(name="qkv", bufs=1) as qkvp,
        ):
            wo_sb = constp.tile([128, 2 * HID], BF16)
            bq_sb = constp.tile([128, 2], F32)
            # per-head strips, duplicated across both partition bands so a
            # scores chunk-pair runs as two concurrent row-tiled matmuls
            # reading the same free offset
            qt2 = qkvp.tile([128, HPC * S], FP16)
            kt2 = qkvp.tile([128, HPC * S], FP16)
            vt_sb = qkvp.tile([128, 2 * S], BF16)
            v_sb = qkvp.tile([128, TC * HPC * 128], BF16)
            ctxf_sb = qkvp.tile([128, 2 * S], BF16)

            ones_ap = v_sb[:, :].rearrange("p (s e) -> p s e", e=128)[:, :, HD:HD + 1]
            nc.vector.memset(ones_ap, 1.0)

            # ---- phase A: projections, V(ci) -> Q(ci) -> K(ci) -----------
            with (
                tc.tile_pool(name="xw", bufs=1) as xwp,
                tc.tile_pool(name="ps1", bufs=2, space="PSUM") as ps1,
            ):
                xT_sb = xwp.tile([128, HC * S], FP16)
                wq_sb = xwp.tile([128, HC * QC], FP16)
                wk_sb = xwp.tile([128, HC * QC], FP16)
                wv_sb = xwp.tile([128, HC * QC], FP16)

                xt_dmas = {}
                for hc in range(HC):
                    r = slice(hc * 128, (hc + 1) * 128)
                    eng = nc.sync if hc % 2 == 0 else nc.scalar
                    if hc == 0:
                        for j in range(TB):
                            xt_dmas[hc] = eng.dma_start(
                                xT_sb[:, j * 512:(j + 1) * 512],
                                xT[r, j * 512:(j + 1) * 512])
                    else:
                        xt_dmas[hc] = eng.dma_start(
                            xT_sb[:, hc * S:(hc + 1) * S], xT[r, :])
                    nc.scalar.dma_start(wv_sb[:, hc * QC:(hc + 1) * QC],
                                        wv[r, :])
                    nc.sync.dma_start(wq_sb[:, hc * QC:(hc + 1) * QC],
                                      wq[r, :])
                    nc.sync.dma_start(wk_sb[:, hc * QC:(hc + 1) * QC],
                                      wk[r, :])
                for ci in range(2):
                    nc.sync.dma_start(bq_sb[:, ci:ci + 1],
                                      bq[ci * 128:(ci + 1) * 128])

                def emit_proj(w_sb, ci):
                    ps = ps1.tile([128, S], F32, tag="ps1")
                    mms = []
                    for hc in range(HC):
                        for j in range(TB):
                            mms.append(nc.tensor.matmul(
                                ps[:, j * 512:(j + 1) * 512],
                                w_sb[:, hc * QC + ci * 128:
                                     hc * QC + ci * 128 + 128],
                                xT_sb[:, hc * S + j * 512:
                                      hc * S + j * 512 + 512],
                                start=(hc == 0), stop=(hc == HC - 1)))
                    return ps, mms

                v_mms = {}
                for ci in range(2):
                    cs = slice(ci * S, (ci + 1) * S)
                    # V first so its transposes overlap Q/K
                    ps, v_mms[ci] = emit_proj(wv_sb, ci)
                    nc.vector.tensor_copy(vt_sb[:, cs], ps[:, :])
                    # V^T -> natural V: this ci's two heads, split across
                    # both hwdge queues (even t on sync, odd t on scalar)
                    for half in range(2):
                        h = 2 * ci + half
                        for t in range(TC):
                            eng = nc.sync if t % 2 == 0 else nc.scalar
                            eng.dma_start_transpose(
                                v_sb[:, (t * HPC + h) * 128:
                                     (t * HPC + h) * 128 + HD],
                                vt_sb[half * 64:half * 64 + 64,
                                      ci * S + t * 128:ci * S + (t + 1) * 128])

                    hA, hB = 2 * ci, 2 * ci + 1
                    for w_sb, dst, with_bias in ((wq_sb, qt2, True),
                                                 (wk_sb, kt2, False)):
                        ps, _ = emit_proj(w_sb, ci)
                        sA = slice(hA * S, (hA + 1) * S)
                        sB = slice(hB * S, (hB + 1) * S)
                        if with_bias:
                            nc.vector.tensor_scalar_add(
                                dst[0:64, sA], ps[0:64, :],
                                bq_sb[0:64, ci:ci + 1])
                            nc.vector.tensor_scalar_add(
                                dst[64:128, sB], ps[64:128, :],
                                bq_sb[64:128, ci:ci + 1])
                        else:
                            nc.vector.tensor_copy(dst[0:64, sA], ps[0:64, :])
                            nc.vector.tensor_copy(dst[64:128, sB],
                                                  ps[64:128, :])
                        nc.gpsimd.dma_start(dst[64:128, sA], dst[0:64, sA])
                        nc.gpsimd.dma_start(dst[0:64, sB], dst[64:128, sB])

                for hc in range(2, HC):
                    tile.add_dep_helper(xt_dmas[hc].ins,
                                        v_mms[0][4 * (hc - 2) + 3].ins,
                                        reason="pace xT load")
                for ci in range(2):
                    d = nc.scalar.dma_start(
                        wo_sb[:, ci * HID:(ci + 1) * HID],
                        wo[ci * 128:(ci + 1) * 128, :])
                    tile.add_dep_helper(d.ins, v_mms[1][8 + 4 * ci].ins,
                                        reason="pace wo load")

            # ---- phase B: attention, j-outer pipeline --------------------
            with (
                tc.tile_pool(name="bigps", bufs=2, space="PSUM") as bigps,
                tc.tile_pool(name="ctps", bufs=4, space="PSUM") as ctps,
                tc.tile_pool(name="probs", bufs=5) as probsp,
                tc.tile_pool(name="div", bufs=2) as divp,
                tc.tile_pool(name="ostg", bufs=3) as ostg,
            ):
                stages = [(j, h, cp) for j in range(TB) for h in range(HPC)
                          for cp in range(NCP)]
                probs_tiles = {}
                ctx_tiles = {}

                def emit_scores(j, h, cp):
                    hS = h * S
                    c0, c1 = 2 * cp, 2 * cp + 1
                    sp = bigps.tile([128, 1024], F32, tag="big")
                    probs_c = probsp.tile([128, 1024], BF16, tag="probs",
                                          name=f"probs_j{j}h{h}cp{cp}")
                    probs_tiles[(j, h, cp)] = probs_c
                    nc.tensor.matmul(
                        sp[:, 0:512],
                        kt2[0:64, hS + c0 * 128:hS + (c0 + 1) * 128],
                        qt2[0:64, hS + j * 512:hS + (j + 1) * 512],
                        start=True, stop=True)
                    nc.tensor.matmul(
                        sp[:, 512:1024],
                        kt2[64:128, hS + c1 * 128:hS + (c1 + 1) * 128],
                        qt2[64:128, hS + j * 512:hS + (j + 1) * 512],
                        start=True, stop=True)
                    if cp in SCH_CPS:
                        nc.vector.tensor_scalar(
                            probs_c[:, :].bitcast(I16), sp[:, :],
                            SCH_A, SCH_B, op0=MULT, op1=ADD)
                    else:
                        nc.scalar.activation(probs_c[:, :], sp[:, :], EXP)

                def emit_ctx(j, h, cp):
                    if cp == 0:
                        ctx_tiles[(j, h)] = (
                            ctps.tile([65, 512], F32, tag="ctx",
                                      name=f"ctxA_j{j}h{h}"),
                            ctps.tile([65, 512], F32, tag="ctx",
                                      name=f"ctxB_j{j}h{h}"))
                    ctx_ab = ctx_tiles[(j, h)]
                    probs_c = probs_tiles.pop((j, h, cp))
                    for ck in range(2):
                        c = 2 * cp + ck
                        vbase = (c * HPC + h) * 128
                        nc.tensor.matmul(
                            ctx_ab[ck][0:65, :],
                            v_sb[:, vbase:vbase + 65],
                            probs_c[:, ck * 512:(ck + 1) * 512],
                            start=(cp == 0), stop=(cp == NCP - 1))

                def emit_division(j, h):
                    ci, half = h // 2, h % 2
                    ctxA, ctxB = ctx_tiles.pop((j, h))
                    crawA = divp.tile([65, 512], F32, tag="crawA")
                    nc.vector.tensor_copy(crawA[0:65, :], ctxA[0:65, :])
                    craw = divp.tile([65, 512], F32, tag="craw")
                    nc.vector.tensor_tensor(out=craw[0:65, :],
                                            in0=ctxB[0:65, :],
                                            in1=crawA[0:65, :], op=TADD)
                    denr = divp.tile([128, 4], F32, tag="denr")
                    nc.gpsimd.dma_start(denr[:, :], craw[64:65, :])
                    recr = divp.tile([128, 4], F32, tag="recr")
                    nc.vector.reciprocal(recr[:], denr[:])
                    rrow = divp.tile([1, 512], F32, tag="rrow")
                    nc.gpsimd.dma_start(rrow[:, :], recr[:, :])
                    Dt = divp.tile([64, 512], F32, tag="Dt")
                    nc.gpsimd.partition_broadcast(Dt[:, :], rrow[0:1, :])
                    dst_cols = slice(ci * S + j * 512, ci * S + (j + 1) * 512)
                    if half == 0:
                        nc.vector.tensor_tensor(
                            out=ctxf_sb[0:64, dst_cols],
                            in0=craw[0:64, :], in1=Dt[0:64, :], op=MULT)
                    else:
                        ctxd = divp.tile([64, 512], BF16, tag="ctxd")
                        nc.vector.tensor_tensor(
                            out=ctxd[0:64, :],
                            in0=craw[0:64, :], in1=Dt[0:64, :], op=MULT)
                        nc.scalar.dma_start(ctxf_sb[64:128, dst_cols],
                                            ctxd[0:64, :])

                def emit_outproj_t(t):
                    op = bigps.tile([128, 1024], F32, tag="big")
                    for ci in range(2):
                        for oc in range(2):
                            nc.tensor.matmul(
                                op[:, oc * 512:(oc + 1) * 512],
                                ctxf_sb[:, ci * S + t * 128:
                                        ci * S + t * 128 + 128],
                                wo_sb[:, ci * HID + oc * 512:
                                      ci * HID + oc * 512 + 512],
                                start=(ci == 0), stop=(ci == 1))
                    ot = ostg.tile([128, 1024], BF16, tag="ot")
                    nc.scalar.copy(ot[:, 0:512], op[:, 0:512])
                    nc.scalar.copy(ot[:, 512:1024], op[:, 512:1024])
                    nc.sync.dma_start(out[t * 128:(t + 1) * 128, :], ot[:, :])

                LEAD = 2
                op_due = {}
                nsteps = len(stages) + LEAD + OP_DELAY + 4
                for i in range(nsteps):
                    if i < len(stages):
                        emit_scores(*stages[i])
                    if LEAD <= i < len(stages) + LEAD:
                        j, h, cp = stages[i - LEAD]
                        emit_ctx(j, h, cp)
                        if cp == NCP - 1:
                            emit_division(j, h)
                            if h == HPC - 1:
                                for tt in range(4):
                                    op_due.setdefault(
                                        i + OP_DELAY + tt, []).append(4 * j + tt)
                    for t in op_due.pop(i, ()):
                        emit_outproj_t(t)

    nc.compile()
    return nc


_NC = None


def _get_nc():
    global _NC
    if _NC is None:
        _NC = build_nc()
    return _NC


def make_in_maps(x, Wq, bq, Wk, bk, Wv, bv, Wo, bo):
    in_maps = []
    for core in range(NCORES):
        b, g = core // 4, core % 4
        sl = slice(g * QC, (g + 1) * QC)
        in_maps.append({
            "xT": np.ascontiguousarray(x[b].T).astype(np.float16),
            "wq": (np.ascontiguousarray(Wq[:, sl]) * 0.125).astype(np.float16),
            "wk": np.ascontiguousarray(Wk[:, sl]).astype(np.float16),
            "wv": np.ascontiguousarray(Wv[:, sl]).astype(np.float16),
            "wo": np.ascontiguousarray(Wo[sl, :]).astype(ml_dtypes.bfloat16),
            "bq": (np.asarray(bq[sl]) * 0.125).astype(np.float32),
        })
    return in_maps


def combine_outputs(core_outs, Wv_bias_term):
    full = np.empty((B, S, HID), np.float32)
    for b in range(B):
        acc = core_outs[4 * b].astype(np.float32).copy()
        for g in range(1, 4):
            acc += core_outs[4 * b + g]
        full[b] = acc + Wv_bias_term
    return full


def kernel(**inputs):
    x = np.asarray(inputs["x"], np.float32)
    Wq = np.asarray(inputs["Wq"], np.float32)
    bq = np.asarray(inputs["bq"], np.float32)
    Wk = np.asarray(inputs["Wk"], np.float32)
    bk = np.asarray(inputs["bk"], np.float32)
    Wv = np.asarray(inputs["Wv"], np.float32)
    bv = np.asarray(inputs["bv"], np.float32)
    Wo = np.asarray(inputs["Wo"], np.float32)
    bo = np.asarray(inputs["bo"], np.float32)

    nc = _get_nc()
    in_maps = make_in_maps(x, Wq, bq, Wk, bk, Wv, bv, Wo, bo)
    res = run_bass_kernel_spmd(nc, in_maps, core_ids=list(range(NCORES)))
    core_outs = [res.results[c]["out"] for c in range(NCORES)]
    bias_term = (bv @ Wo + bo).astype(np.float32)
    return combine_outputs(core_outs, bias_term)


# revision 8
# speedup vs baseline: 1.7005x; 1.1244x over previous
"""Multi-head attention (B=2, S=2048, H=1024, 16 heads) on 8 TRN2 NeuronCores.

Sharding: tensor-parallel heads x data-parallel batch. Core c -> batch c//4,
head group c%4 (4 heads each). Megatron-style partial out-projections summed
on the host.

V4 design:
- Natural V computed directly: per 128-token chunk, stationary = xT chunk
  [128 hid, 128 tok], moving = Wv [128 hid, 256] accumulated over hidden
  chunks -> [128 tok, 256] PSUM, then one strided DVE copy into the
  per-(t, h) V strips. No V^T staging, no transposes.
- Q^T/K^T per-head strips duplicated across both partition bands (the
  concurrent row-tiled scores pair must read both bands at the SAME free
  offset of the same tile -- split tiles raced on real hardware).
- Phase A order Q(ci0), K(ci0), V, Q(ci1), K(ci1): scores for head-pair 0
  start ~16us in; V strips are produced t-ascending just ahead of the first
  ctx consumption.
- j-outer attention pipeline per 512-query block, heads ordered (1,3,0,2)
  so the last division before each out-projection is an even head (no
  cross-band DMA hop on the critical path). ctx accumulates alternately
  into two PSUM banks (A: even chunks, B: odd chunks) to dodge same-bank
  accumulation turnaround; division does craw = A + B which frees both.
- Softmax denominator via the ones-column row-64 trick; reciprocal on a
  [128, 4] DMA-gathered layout.
- exp split: ACT 5/8 of tiles (real exp), DVE 3/8 via Schraudolph fast-exp
  (int16 bits = s*a + b bitcast as bf16, bias tuned so the mean
  multiplicative error matches exp for consistent mixing).
- bk dropped: it shifts every score of a query equally and softmax cancels
  it. bv/bo are folded into a host-side additive constant; bq applied
  on-device.
"""

import ml_dtypes
import numpy as np

import concourse.bacc as bacc
import concourse.mybir as mybir
import concourse.tile as tile
from concourse.bass_utils import run_bass_kernel_spmd

NCORES = 8
B, S, HID = 2, 2048, 1024
NH, HD = 16, 64
HPC = 4            # heads per core
QC = HPC * HD      # 256 local projection cols per core
HC = HID // 128    # 8 hidden chunks
TC = S // 128      # 16 token chunks
TB = S // 512      # 4 query blocks
NCP = TC // 2      # 8 chunk-pairs per head

F32 = mybir.dt.float32
BF16 = mybir.dt.bfloat16
FP16 = mybir.dt.float16
I16 = mybir.dt.int16
EXP = mybir.ActivationFunctionType.Exp
MULT = mybir.AluOpType.mult
ADD = mybir.AluOpType.add

SCH_A = 128.0 / float(np.log(2.0))
SCH_B = 16256.0 - 7.0
SCH_CPS = (2, 5, 7)
OP_DELAY = 10
HEAD_ORDER = (1, 3, 0, 2)


def build_nc():
    nc = bacc.Bacc("TRN2", target_bir_lowering=False, debug=False,
                   num_devices=NCORES)
    xT = nc.declare_dram_parameter("xT", [HID, S], FP16, isOutput=False)
    wq = nc.declare_dram_parameter("wq", [HID, QC], FP16, isOutput=False)
    wk = nc.declare_dram_parameter("wk", [HID, QC], FP16, isOutput=False)
    wv = nc.declare_dram_parameter("wv", [HID, QC], FP16, isOutput=False)
    wo = nc.declare_dram_parameter("wo", [QC, HID], BF16, isOutput=False)
    bq = nc.declare_dram_parameter("bq", [QC], F32, isOutput=False)
    out = nc.declare_dram_parameter("out", [S, HID], BF16, isOutput=True)

    with tile.TileContext(nc) as tc:
        with (
            tc.tile_pool(name="const", bufs=1) as constp,
            tc.tile_pool(name="qkv", bufs=1) as qkvp,
        ):
            wo_sb = constp.tile([128, 2 * HID], BF16)
            bq_sb = constp.tile([128, 2], F32)
            # per-head strips, duplicated across both partition bands
            qt2 = qkvp.tile([128, HPC * S], FP16)
            kt2 = qkvp.tile([128, HPC * S], FP16)
            # natural V strips: strip (t*HPC + h) at col offset *128,
            # V in cols 0:64, ones in col 64
            v_sb = qkvp.tile([128, TC * HPC * 128], BF16)
            ctxf_sb = qkvp.tile([128, 2 * S], BF16)

            ones_ap = v_sb[:, :].rearrange("p (s e) -> p s e", e=128)[:, :, HD:HD + 1]
            nc.vector.memset(ones_ap, 1.0)

            # ---- phase A: projections --------------------------------
            with (
                tc.tile_pool(name="xw", bufs=1) as xwp,
                tc.tile_pool(name="ps1", bufs=3, space="PSUM") as ps1,
                tc.tile_pool(name="vps", bufs=2, space="PSUM") as vps,
            ):
                xT_sb = xwp.tile([128, HC * S], FP16)
                wq_sb = xwp.tile([128, HC * QC], FP16)
                wk_sb = xwp.tile([128, HC * QC], FP16)
                wv_sb = xwp.tile([128, HC * QC], FP16)

                xt_dmas = {}
                for hc in range(HC):
                    r = slice(hc * 128, (hc + 1) * 128)
                    eng = nc.sync if hc % 2 == 0 else nc.scalar
                    if hc == 0:
                        for j in range(TB):
                            xt_dmas[hc] = eng.dma_start(
                                xT_sb[:, j * 512:(j + 1) * 512],
                                xT[r, j * 512:(j + 1) * 512])
                    else:
                        xt_dmas[hc] = eng.dma_start(
                            xT_sb[:, hc * S:(hc + 1) * S], xT[r, :])
                    nc.scalar.dma_start(wq_sb[:, hc * QC:(hc + 1) * QC],
                                        wq[r, :])
                    nc.sync.dma_start(wk_sb[:, hc * QC:(hc + 1) * QC],
                                      wk[r, :])
                    nc.scalar.dma_start(wv_sb[:, hc * QC:(hc + 1) * QC],
                                        wv[r, :])
                for ci in range(2):
                    nc.sync.dma_start(bq_sb[:, ci:ci + 1],
                                      bq[ci * 128:(ci + 1) * 128])

                qk_mms = {}

                def emit_qk(name, w_sb, dst, ci, with_bias):
                    hA, hB = 2 * ci, 2 * ci + 1
                    for jh in range(2):
                        ps = ps1.tile([128, 1024], F32, tag="ps1")
                        for hc in range(HC):
                            for jj in range(2):
                                j = 2 * jh + jj
                                qk_mms[(name, ci, hc, j)] = nc.tensor.matmul(
                                    ps[:, jj * 512:(jj + 1) * 512],
                                    w_sb[:, hc * QC + ci * 128:
                                         hc * QC + ci * 128 + 128],
                                    xT_sb[:, hc * S + j * 512:
                                          hc * S + j * 512 + 512],
                                    start=(hc == 0), stop=(hc == HC - 1))
                        cols = slice(2 * jh * 512, 2 * (jh + 1) * 512)
                        sA = slice(hA * S + 2 * jh * 512,
                                   hA * S + 2 * (jh + 1) * 512)
                        sB = slice(hB * S + 2 * jh * 512,
                                   hB * S + 2 * (jh + 1) * 512)
                        if with_bias:
                            nc.vector.tensor_scalar_add(
                                dst[0:64, sA], ps[0:64, :],
                                bq_sb[0:64, ci:ci + 1])
                            nc.vector.tensor_scalar_add(
                                dst[64:128, sB], ps[64:128, :],
                                bq_sb[64:128, ci:ci + 1])
                        else:
                            nc.vector.tensor_copy(dst[0:64, sA], ps[0:64, :])
                            nc.vector.tensor_copy(dst[64:128, sB],
                                                  ps[64:128, :])
                    sA = slice(hA * S, (hA + 1) * S)
                    sB = slice(hB * S, (hB + 1) * S)
                    nc.gpsimd.dma_start(dst[64:128, sA], dst[0:64, sA])
                    nc.gpsimd.dma_start(dst[0:64, sB], dst[64:128, sB])

                def emit_v():
                    for t in range(TC):
                        vt = vps.tile([128, 256], F32, tag="vt")
                        for hc in range(HC):
                            nc.tensor.matmul(
                                vt[:, :],
                                xT_sb[:, hc * S + t * 128:
                                      hc * S + (t + 1) * 128],
                                wv_sb[:, hc * QC:(hc + 1) * QC],
                                start=(hc == 0), stop=(hc == HC - 1))
                        dst = v_sb[:, t * 512:(t + 1) * 512].rearrange(
                            "p (h e) -> p h e", e=128)[:, :, 0:HD]
                        src = vt[:, :].rearrange("p (h e) -> p h e", e=HD)
                        nc.vector.tensor_copy(dst, src)

                emit_qk("q", wq_sb, qt2, 0, True)
                emit_qk("k", wk_sb, kt2, 0, False)
                emit_v()
                emit_qk("q", wq_sb, qt2, 1, True)
                emit_qk("k", wk_sb, kt2, 1, False)

                for hc in range(2, HC):
                    tile.add_dep_helper(xt_dmas[hc].ins,
                                        qk_mms[("q", 0, hc - 2, 3)].ins,
                                        reason="pace xT load")
                for ci in range(2):
                    d = nc.sync.dma_start(
                        wo_sb[:, ci * HID:(ci + 1) * HID],
                        wo[ci * 128:(ci + 1) * 128, :])
                    tile.add_dep_helper(d.ins, qk_mms[("k", 0, 5 + ci, 1)].ins,
                                        reason="pace wo load")

            # ---- phase B: attention, j-outer pipeline --------------------
            with (
                tc.tile_pool(name="bigps", bufs=2, space="PSUM") as bigps,
                tc.tile_pool(name="ctps", bufs=4, space="PSUM") as ctps,
                tc.tile_pool(name="probs", bufs=5) as probsp,
                tc.tile_pool(name="div", bufs=2) as divp,
                tc.tile_pool(name="ostg", bufs=3) as ostg,
            ):
                stages = [(j, h, cp) for j in range(TB) for h in HEAD_ORDER
                          for cp in range(NCP)]
                probs_tiles = {}
                ctx_tiles = {}

                def emit_scores(j, h, cp):
                    hS = h * S
                    c0, c1 = 2 * cp, 2 * cp + 1
                    sp = bigps.tile([128, 1024], F32, tag="big")
                    probs_c = probsp.tile([128, 1024], BF16, tag="probs",
                                          name=f"probs_j{j}h{h}cp{cp}")
                    probs_tiles[(j, h, cp)] = probs_c
                    nc.tensor.matmul(
                        sp[:, 0:512],
                        kt2[0:64, hS + c0 * 128:hS + (c0 + 1) * 128],
                        qt2[0:64, hS + j * 512:hS + (j + 1) * 512],
                        start=True, stop=True)
                    nc.tensor.matmul(
                        sp[:, 512:1024],
                        kt2[64:128, hS + c1 * 128:hS + (c1 + 1) * 128],
                        qt2[64:128, hS + j * 512:hS + (j + 1) * 512],
                        start=True, stop=True)
                    if cp in SCH_CPS:
                        nc.vector.tensor_scalar(
                            probs_c[:, :].bitcast(I16), sp[:, :],
                            SCH_A, SCH_B, op0=MULT, op1=ADD)
                    else:
                        nc.scalar.activation(probs_c[:, :], sp[:, :], EXP)

                def emit_ctx(j, h, cp):
                    if cp == 0:
                        ctx_tiles[(j, h)] = (
                            ctps.tile([65, 512], F32, tag="ctx",
                                      name=f"ctxA_j{j}h{h}"),
                            ctps.tile([65, 512], F32, tag="ctx",
                                      name=f"ctxB_j{j}h{h}"))
                    ctx_ab = ctx_tiles[(j, h)]
                    probs_c = probs_tiles.pop((j, h, cp))
                    for ck in range(2):
                        c = 2 * cp + ck
                        vbase = (c * HPC + h) * 128
                        nc.tensor.matmul(
                            ctx_ab[ck][0:65, :],
                            v_sb[:, vbase:vbase + 65],
                            probs_c[:, ck * 512:(ck + 1) * 512],
                            start=(cp == 0), stop=(cp == NCP - 1))

                def emit_division(j, h):
                    ci, half = h // 2, h % 2
                    ctxA, ctxB = ctx_tiles.pop((j, h))
                    crawA = divp.tile([65, 512], F32, tag="crawA")
                    nc.vector.tensor_copy(crawA[0:65, :], ctxA[0:65, :])
                    craw = divp.tile([65, 512], F32, tag="craw")
                    nc.vector.tensor_tensor(out=craw[0:65, :],
                                            in0=ctxB[0:65, :],
                                            in1=crawA[0:65, :], op=ADD)
                    denr = divp.tile([128, 4], F32, tag="denr")
                    nc.gpsimd.dma_start(denr[:, :], craw[64:65, :])
                    recr = divp.tile([128, 4], F32, tag="recr")
                    nc.vector.reciprocal(recr[:], denr[:])
                    rrow = divp.tile([1, 512], F32, tag="rrow")
                    nc.gpsimd.dma_start(rrow[:, :], recr[:, :])
                    Dt = divp.tile([64, 512], F32, tag="Dt")
                    nc.gpsimd.partition_broadcast(Dt[:, :], rrow[0:1, :])
                    dst_cols = slice(ci * S + j * 512, ci * S + (j + 1) * 512)
                    if half == 0:
                        nc.vector.tensor_tensor(
                            out=ctxf_sb[0:64, dst_cols],
                            in0=craw[0:64, :], in1=Dt[0:64, :], op=MULT)
                    else:
                        ctxd = divp.tile([64, 512], BF16, tag="ctxd")
                        nc.vector.tensor_tensor(
                            out=ctxd[0:64, :],
                            in0=craw[0:64, :], in1=Dt[0:64, :], op=MULT)
                        nc.scalar.dma_start(ctxf_sb[64:128, dst_cols],
                                            ctxd[0:64, :])

                def emit_outproj_t(t):
                    op = bigps.tile([128, 1024], F32, tag="big")
                    for ci in range(2):
                        for oc in range(2):
                            nc.tensor.matmul(
                                op[:, oc * 512:(oc + 1) * 512],
                                ctxf_sb[:, ci * S + t * 128:
                                        ci * S + t * 128 + 128],
                                wo_sb[:, ci * HID + oc * 512:
                                      ci * HID + oc * 512 + 512],
                                start=(ci == 0), stop=(ci == 1))
                    ot = ostg.tile([128, 1024], BF16, tag="ot")
                    nc.scalar.copy(ot[:, 0:512], op[:, 0:512])
                    nc.scalar.copy(ot[:, 512:1024], op[:, 512:1024])
                    nc.sync.dma_start(out[t * 128:(t + 1) * 128, :], ot[:, :])

                LEAD = 2
                op_due = {}
                nsteps = len(stages) + LEAD + OP_DELAY + 4
                for i in range(nsteps):
                    if i < len(stages):
                        emit_scores(*stages[i])
                    if LEAD <= i < len(stages) + LEAD:
                        j, h, cp = stages[i - LEAD]
                        emit_ctx(j, h, cp)
                        if cp == NCP - 1:
                            emit_division(j, h)
                            if h == HEAD_ORDER[-1]:
                                for tt in range(4):
                                    op_due.setdefault(
                                        i + OP_DELAY + tt, []).append(4 * j + tt)
                    for t in op_due.pop(i, ()):
                        emit_outproj_t(t)

    nc.compile()
    return nc


_NC = None


def _get_nc():
    global _NC
    if _NC is None:
        _NC = build_nc()
    return _NC


def make_in_maps(x, Wq, bq, Wk, bk, Wv, bv, Wo, bo):
    in_maps = []
    for core in range(NCORES):
        b, g = core // 4, core % 4
        sl = slice(g * QC, (g + 1) * QC)
        in_maps.append({
            "xT": np.ascontiguousarray(x[b].T).astype(np.float16),
            "wq": (np.ascontiguousarray(Wq[:, sl]) * 0.125).astype(np.float16),
            "wk": np.ascontiguousarray(Wk[:, sl]).astype(np.float16),
            "wv": np.ascontiguousarray(Wv[:, sl]).astype(np.float16),
            "wo": np.ascontiguousarray(Wo[sl, :]).astype(ml_dtypes.bfloat16),
            "bq": (np.asarray(bq[sl]) * 0.125).astype(np.float32),
        })
    return in_maps


def combine_outputs(core_outs, Wv_bias_term):
    full = np.empty((B, S, HID), np.float32)
    for b in range(B):
        acc = core_outs[4 * b].astype(np.float32).copy()
        for g in range(1, 4):
            acc += core_outs[4 * b + g]
        full[b] = acc + Wv_bias_term
    return full


def kernel(**inputs):
    x = np.asarray(inputs["x"], np.float32)
    Wq = np.asarray(inputs["Wq"], np.float32)
    bq = np.asarray(inputs["bq"], np.float32)
    Wk = np.asarray(inputs["Wk"], np.float32)
    bk = np.asarray(inputs["bk"], np.float32)
    Wv = np.asarray(inputs["Wv"], np.float32)
    bv = np.asarray(inputs["bv"], np.float32)
    Wo = np.asarray(inputs["Wo"], np.float32)
    bo = np.asarray(inputs["bo"], np.float32)

    nc = _get_nc()
    in_maps = make_in_maps(x, Wq, bq, Wk, bk, Wv, bv, Wo, bo)
    res = run_bass_kernel_spmd(nc, in_maps, core_ids=list(range(NCORES)))
    core_outs = [res.results[c]["out"] for c in range(NCORES)]
    bias_term = (bv @ Wo + bo).astype(np.float32)
    return combine_outputs(core_outs, bias_term)
